# revision 18
# baseline (speedup 1.0000x reference)
"""Trainium2 Bass kernel for nn_ATC_Network (2-layer GCN + BN + LeakyReLU).

Computation (see reference):
    row, col, w  (+ self loops w=1)
    deg[c] = sum_{e: col=c} w_e ;  dis = rsqrt(deg)
    norm_e = dis[row]*w*dis[col]
    z1[c]  = sum_e norm_e * x[row]          (conv1 aggregate, incl self loop)
    y1     = z1 @ W1 + b1 ; x2 = LeakyReLU(BN(y1))
    z2[c]  = sum_e norm_e * x2[row]
    y2     = z2 @ W2 + b2 ; out = BN(y2)

v2 design notes:
  - All graph normalization (deg, dis, norm) is computed host-side in
    preprocess(); self-loops become ordinary edges.  The device sees only
    padded slot streams + block tables.
  - x is cast to f16 host-side; conv1 gathers 256B rows straight from it.
  - The weighted one-hot ws ([128, NB, Cu] f16) is built once on-device
    (3 DVE passes) and shared by both conv layers.
  - conv2 pre-folds W2: aggregates u2 = x2@W2 (64-wide).  u2 is
    AllGather'd as f16 [N, 64] (6.4MB) and gathered in PAIRS (256B = two
    nodes); an even/odd split of ws (built per group) selects the right
    half of each gathered pair via two matmuls per block.
  - BN stats via second-moment matrices (PE) + AllReduce; the moment
    AllReduce overlaps the y1 = z1@W1 matmul.
"""

import sys

sys.path.insert(0, "/opt/trn_rl_repo")

import numpy as np
import ml_dtypes

import concourse.bass as bass
import concourse.tile as tile
from concourse import bacc, bass_utils, mybir
from concourse.masks import make_identity

FP32 = mybir.dt.float32
F16 = mybir.dt.float16
I16 = mybir.dt.int16

# ---------------------------------------------------------------- config ----
CFG = dict(
    N=50000, E=800000, F=128, H=128, O=64, NCORE=8,
    HALF=25000,          # src split point for int16 gather indices
    GT=4,                # dest tiles per gather group
    YC=512,              # node columns per y-matmul chunk
    EPS=1e-5, NEG=0.01,
)


# ---------------------------------------------------------- preprocessing ---
def preprocess(adj, w, cfg):
    """Host-side graph prep: norm, self-loops, per-core padded slot streams.

    Returns block tables with *uniform* shapes across cores (SPMD: one
    instruction stream; per-core data differs).
    """
    N, E, NCORE, HALF = cfg["N"], cfg["E"], cfg["NCORE"], cfg["HALF"]
    NP = N // NCORE
    T = (NP + 127) // 128
    LT = NP - 128 * (T - 1)

    row0 = np.asarray(adj[0], np.int64)
    col0 = np.asarray(adj[1], np.int64)
    w = np.asarray(w, np.float32)

    # gcn_norm host-side (incl self loops, weight 1)
    deg = np.zeros(N, np.float64)
    np.add.at(deg, col0, w.astype(np.float64))
    deg += 1.0
    dis = (1.0 / np.sqrt(deg)).astype(np.float32)

    row = np.concatenate([row0, np.arange(N, dtype=np.int64)])
    col = np.concatenate([col0, np.arange(N, dtype=np.int64)])
    nrm = np.concatenate([dis[row0] * w * dis[col0], dis * dis]).astype(np.float32)
    EA = E + N

    core = col // NP
    lc = col % NP
    tl = lc // 128
    dl = lc % 128
    half = (row >= HALF).astype(np.int64)

    # stable sort by (core, half, tile, dest)
    key = ((core * 2 + half) * T + tl) * 128 + dl
    order = np.argsort(key, kind="stable")

    # caps per (half, tile): max over cores, ceil to 128
    cht = (core * 2 + half) * T + tl
    cnt_cht = np.bincount(cht, minlength=NCORE * 2 * T).reshape(NCORE, 2, T)
    cap = np.maximum(cnt_cht.max(0), 1)                     # [2, T]
    cap = ((cap + 127) // 128) * 128
    tsb = np.zeros((2, T + 1), np.int64)                    # slot base per tile
    tsb[:, 1:] = np.cumsum(cap, 1)
    L = tsb[:, -1].copy()                                   # stream length/half
    NBA, NBB = int(L[0] // 128), int(L[1] // 128)
    NB = NBA + NBB

    # per-edge rank within (core, half, tile) -> stream slot
    sk2 = cht[order]
    g2_start = np.r_[0, np.flatnonzero(np.diff(sk2)) + 1]
    g2_id = np.cumsum(np.r_[0, np.diff(sk2) != 0])
    rank_in_ct = np.arange(EA) - g2_start[g2_id]
    slot = tsb[half[order], tl[order]] + rank_in_ct          # slot within half-stream

    # fill per-core streams (flat per (core, half))
    oc, oh = core[order], half[order]
    orow, onrm, odl = row[order], nrm[order], dl[order]
    idx1 = [[np.zeros(int(L[h]), np.int32) for h in range(2)] for _ in range(NCORE)]
    idx2 = [[np.zeros(int(L[h]), np.int32) for h in range(2)] for _ in range(NCORE)]
    n_st = [[np.zeros(int(L[h]), np.float32) for h in range(2)] for _ in range(NCORE)]
    d_st = [[np.zeros(int(L[h]), np.int32) for h in range(2)] for _ in range(NCORE)]
    p_st = [[np.zeros(int(L[h]), np.float32) for h in range(2)] for _ in range(NCORE)]
    for c in range(NCORE):
        for h in range(2):
            m = (oc == c) & (oh == h)
            s = slot[m]
            idx1[c][h][s] = orow[m] - h * HALF
            idx2[c][h][s] = orow[m] // 2
            n_st[c][h][s] = onrm[m]
            d_st[c][h][s] = odl[m]
            p_st[c][h][s] = 1.0 - (orow[m] & 1)              # even -> 1

    # uniform block structure: d0 / width per 128-slot block (union over cores)
    d0 = np.zeros((2, max(NBA, NBB)), np.int64)
    dend = np.ones((2, max(NBA, NBB)), np.int64)
    for h, nb in ((0, NBA), (1, NBB)):
        dmin = np.full((nb,), 1 << 30, np.int64)
        dmax = np.full((nb,), -1, np.int64)
        for c in range(NCORE):
            dd = d_st[c][h].reshape(nb, 128)
            ww = n_st[c][h].reshape(nb, 128)
            real = ww > 0
            any_real = real.any(1)
            dmasked = np.where(real, dd, 1 << 30)
            dmin2 = dmasked.min(1)
            dmasked = np.where(real, dd, -1)
            dmax2 = dmasked.max(1)
            dmin = np.where(any_real, np.minimum(dmin, dmin2), dmin)
            dmax = np.where(any_real, np.maximum(dmax, dmax2), dmax)
        none = dmax < 0
        dmin[none] = 0
        dmax[none] = 0
        wid = dmax + 1 - dmin
        d0[h, :nb] = dmin
        dend[h, :nb] = dmin + wid
    Cu = int((dend - d0).max())
    Cu = max(Cu, 2)

    # block -> tile map (per half)
    blk_tile = np.zeros((2, max(NBA, NBB)), np.int64)
    for h in range(2):
        for t in range(T):
            blk_tile[h, tsb[h, t] // 128: tsb[h, t + 1] // 128] = t

    # packed one-hot weight table: block b occupies cols offP[b]:offP[b]+cb_b
    cbs = np.zeros(NB, np.int64)
    cbs[:NBA] = np.minimum(np.minimum(dend[0, :NBA] - d0[0, :NBA], Cu),
                           128 - d0[0, :NBA])
    cbs[NBA:] = np.minimum(np.minimum(dend[1, :NBB] - d0[1, :NBB], Cu),
                           128 - d0[1, :NBB])
    offP = np.zeros(NB + 1, np.int64)
    offP[1:] = np.cumsum(cbs)
    P = int(((offP[-1] + 15) // 16) * 16)
    wsP = np.zeros((NCORE, 128, P), np.float16)
    for c in range(NCORE):
        for h, nb0 in ((0, 0), (1, NBA)):
            L_ = int(L[h])
            sl = np.arange(L_)
            bidx = sl // 128 + nb0
            p = sl % 128
            nrm_s = n_st[c][h]
            dls = d_st[c][h]
            colP = offP[bidx] + dls - (d0[h])[sl // 128]
            ok = nrm_s > 0
            wsP[c][p[ok], colP[ok]] = nrm_s[ok]

    # per-core block tables in [128, NB] layout
    def blockify(streams, dt):
        outa = np.zeros((NCORE, 128, NB), dt)
        for c in range(NCORE):
            outa[c, :, :NBA] = streams[c][0].reshape(NBA, 128).T
            outa[c, :, NBA:] = streams[c][1].reshape(NBB, 128).T
        return outa

    par = blockify(p_st, np.float32).astype(np.float16)
    parO = blockify([[1.0 - x for x in ps] for ps in p_st],
                    np.float32).astype(np.float16)

    # idx arrays, wrapped [16, L/16] then replicated to 128 partitions
    def wrap_idx(a):
        v = a.astype(np.int16).reshape(-1, 16).T  # [16, L/16]
        return np.tile(v, (8, 1))                 # [128, L/16]

    idx1A = np.stack([wrap_idx(idx1[c][0]) for c in range(NCORE)])
    idx1B = np.stack([wrap_idx(idx1[c][1]) for c in range(NCORE)])
    idx2A = np.stack([wrap_idx(idx2[c][0]) for c in range(NCORE)])
    idx2B = np.stack([wrap_idx(idx2[c][1]) for c in range(NCORE)])

    # gather groups (GT tiles each)
    GT = cfg["GT"]
    groups = []
    for g0 in range(0, T, GT):
        g1 = min(g0 + GT, T)
        ent = dict(t0=g0, t1=g1)
        for h, tag in ((0, "A"), (1, "B")):
            s0, s1 = int(tsb[h, g0]), int(tsb[h, g1])
            ent[f"s0{tag}"], ent[f"s1{tag}"] = s0, s1
            ent[f"b0{tag}"], ent[f"b1{tag}"] = s0 // 128, s1 // 128
        groups.append(ent)
    GNBH = max(max(g["b1A"] - g["b0A"], g["b1B"] - g["b0B"]) for g in groups)
    GNB = max((g["b1A"] - g["b0A"]) + (g["b1B"] - g["b0B"]) for g in groups)

    pad_ratio = (L[0] + L[1]) / max(1.0, EA / NCORE)
    return dict(
        cfg=cfg, NP=NP, T=T, LT=LT, NBA=NBA, NBB=NBB, NB=NB, Cu=Cu,
        LA=int(L[0]), LB=int(L[1]), tsb=tsb, d0=d0, dend=dend,
        blk_tile=blk_tile, groups=groups, GNBH=GNBH, GNB=GNB,
        wsP=wsP, offP=offP, cbs=cbs, P=P, par=par, parO=parO,
        idx1A=idx1A, idx1B=idx1B, idx2A=idx2A, idx2B=idx2B,
        pad_ratio=float(pad_ratio),
    )


# ------------------------------------------------------------ bass program --
STAGES = ["ws", "conv1", "bn1", "ag", "conv2", "full"]


def build(st, stage="full", reps=1):
    slev = STAGES.index(stage)
    cfg = st["cfg"]
    N, F, H, O, NCORE = cfg["N"], cfg["F"], cfg["H"], cfg["O"], cfg["NCORE"]
    HALF, EPS, NEG, YC = cfg["HALF"], cfg["EPS"], cfg["NEG"], cfg["YC"]
    NP, T, LT, NB, NBA, NBB, Cu = (st["NP"], st["T"], st["LT"], st["NB"],
                                   st["NBA"], st["NBB"], st["Cu"])
    d0s, dends, tsb = st["d0"], st["dend"], st["tsb"]
    GNBH, GNB = st["GNBH"], st["GNB"]
    P, offP, cbs = st["P"], st["offP"], st["cbs"]
    NH = N // 2  # pair count for conv2 gathers
    rg = [list(range(NCORE))]

    nc = bacc.Bacc("TRN2", target_bir_lowering=False, debug=False,
                   num_devices=NCORE, num_swdge_queues=4)
    NQ = 4
    qctr = [0]  # rotate swdge queues so desc-gen overlaps draining

    # --- I/O ---
    x16_d = nc.dram_tensor("x16", [N, F], F16, kind="ExternalInput")
    W1 = nc.dram_tensor("w1", [F, H], FP32, kind="ExternalInput")
    g1 = nc.dram_tensor("g1", [H], FP32, kind="ExternalInput")
    be1 = nc.dram_tensor("beta1", [H], FP32, kind="ExternalInput")
    W2 = nc.dram_tensor("w2m", [H, O], FP32, kind="ExternalInput")
    g2 = nc.dram_tensor("g2", [O], FP32, kind="ExternalInput")
    be2 = nc.dram_tensor("beta2", [O], FP32, kind="ExternalInput")
    i1A_d = nc.dram_tensor("idx1A", [128, st["LA"] // 16], I16, kind="ExternalInput")
    i1B_d = nc.dram_tensor("idx1B", [128, st["LB"] // 16], I16, kind="ExternalInput")
    i2A_d = nc.dram_tensor("idx2A", [128, st["LA"] // 16], I16, kind="ExternalInput")
    i2B_d = nc.dram_tensor("idx2B", [128, st["LB"] // 16], I16, kind="ExternalInput")
    wsP_d = nc.dram_tensor("wsP", [128, P], F16, kind="ExternalInput")
    par_d = nc.dram_tensor("par", [128, NB], F16, kind="ExternalInput")
    parO_d = nc.dram_tensor("parO", [128, NB], F16, kind="ExternalInput")
    out_d = nc.dram_tensor("out", [NP, O], FP32, kind="ExternalOutput")

    def bcast_inner(ap, k):
        return bass.AP(tensor=ap.tensor, offset=ap.offset, ap=ap.ap + [[0, k]])

    def bcast_mid(ap, k):
        # [128, n] -> [128, k, n] with middle stride 0
        return bass.AP(tensor=ap.tensor, offset=ap.offset,
                       ap=[ap.ap[0]] + [[0, k]] + ap.ap[1:])

    def bcast_part(src_ap, off, n, parts=128):
        return bass.AP(tensor=src_ap.tensor, offset=src_ap.offset + off,
                       ap=[[0, parts], [1, n]])

    with tile.TileContext(nc) as tc:
        sing = tc.alloc_tile_pool(name="sing", bufs=1)
        small = tc.alloc_tile_pool(name="small", bufs=3)
        gbufA_p = tc.alloc_tile_pool(name="gbufA", bufs=2)
        gbufB_p = tc.alloc_tile_pool(name="gbufB", bufs=2)
        wseo_p = tc.alloc_tile_pool(name="wseo", bufs=2)
        zpool = tc.alloc_tile_pool(name="zpool", bufs=1)
        ptile = tc.alloc_tile_pool(name="ptile", bufs=2, space="PSUM")
        pmisc = tc.alloc_tile_pool(name="pmisc", bufs=2, space="PSUM")
        pfold = tc.alloc_tile_pool(name="pfold", bufs=1, space="PSUM")
        pmom = tc.alloc_tile_pool(name="pmom", bufs=1, space="PSUM")
        pyc = tc.alloc_tile_pool(name="pyc", bufs=2, space="PSUM")
        dram = tc.alloc_tile_pool(name="dram", bufs=1, space="DRAM")

        # --- persistent DRAM scratch ---
        mo_in = dram.tile([128, H + 1], FP32)
        mo2_in = dram.tile([O, O + 1], FP32)
        u2own_d = dram.tile([NP, O], F16)

        # --- constants ---
        ident = sing.tile([128, 128], FP32)
        make_identity(nc, ident[:])
        ident_h = sing.tile([128, 128], F16)
        nc.vector.tensor_copy(out=ident_h[:], in_=ident[:])
        ones_col = sing.tile([128, 1], FP32)
        nc.vector.memset(ones_col[:], 1.0)
        eps_sb = sing.tile([128, 1], FP32)
        nc.vector.memset(eps_sb[:], EPS)

        W1_sb = sing.tile([F, H], FP32)
        nc.sync.dma_start(out=W1_sb[:], in_=W1[:, :])
        W1_16 = sing.tile([F, H], F16)
        nc.vector.tensor_copy(out=W1_16[:], in_=W1_sb[:])
        W2_sb = sing.tile([H, O], FP32)
        nc.sync.dma_start(out=W2_sb[:], in_=W2[:, :])
        W2_16 = sing.tile([H, O], F16)
        nc.vector.tensor_copy(out=W2_16[:], in_=W2_sb[:])
        g1_sb = sing.tile([H, 1], FP32)
        nc.sync.dma_start(out=g1_sb[:], in_=g1[:, None])
        be1_sb = sing.tile([H, 1], FP32)
        nc.sync.dma_start(out=be1_sb[:], in_=be1[:, None])
        g2_sb = sing.tile([O, 1], FP32)
        nc.sync.dma_start(out=g2_sb[:], in_=g2[:, None])
        be2_sb = sing.tile([O, 1], FP32)
        nc.sync.dma_start(out=be2_sb[:], in_=be2[:, None])

        # --- block metadata ---
        wsP_sb = sing.tile([128, P], F16)
        nc.sync.dma_start(out=wsP_sb[:], in_=wsP_d[:, :])
        par_sb = sing.tile([128, NB], F16)
        nc.sync.dma_start(out=par_sb[:], in_=par_d[:, :])
        parO_sb = sing.tile([128, NB], F16)
        nc.sync.dma_start(out=parO_sb[:], in_=parO_d[:, :])

        def emit_once():
            # per-rep: Shared DRAM must be written by a single instruction
            u2full = dram.tile([N, O], F16, addr_space="Shared", tag=None,
                               uniquify=True, name="u2full")
            mo_out = dram.tile([128, H + 1], FP32, addr_space="Shared",
                               uniquify=True, name="mo_out")
            mo2_out = dram.tile([O, O + 1], FP32, addr_space="Shared",
                                uniquify=True, name="mo2_out")
            # =============== conv layer (shared emitter) ===============
            def conv(layer, zT, HH):
                """Aggregate into zT ([128, NP] f16, rows 0:HH valid); returns
                the AllReduce'd moment tile [128 or HH, HH+1]."""
                Mp = pmom.tile([128, H + 1], FP32, tag="mom")
                for g in st["groups"]:
                    t0, t1 = g["t0"], g["t1"]
                    bufs = {}
                    for h, tg, pool, idx_d in (
                            (0, "A", gbufA_p, i1A_d if layer == 1 else i2A_d),
                            (1, "B", gbufB_p, i1B_d if layer == 1 else i2B_d)):
                        s0, s1 = g[f"s0{tg}"], g[f"s1{tg}"]
                        nb = (s1 - s0) // 128
                        if nb == 0:
                            continue
                        it = small.tile([128, (s1 - s0) // 16], I16, tag=f"idx{tg}")
                        nc.sync.dma_start(out=it[:], in_=idx_d[:, s0 // 16:s1 // 16])
                        gb = pool.tile([128, GNBH, 128], F16, tag=f"g{tg}")
                        if layer == 1:
                            src_ap = (x16_d[0:HALF, :] if h == 0
                                      else x16_d[HALF:N, :])
                        else:
                            # pair view of u2full: [NH, 128] f16
                            u2ap = u2full[:]
                            src_ap = bass.AP(
                                tensor=u2ap.tensor, offset=u2ap.offset,
                                ap=[[128, NH], [1, 128]])
                        GCH = 1024
                        for o in range(0, s1 - s0, GCH):
                            ni = min(GCH, s1 - s0 - o)
                            nc.gpsimd.dma_gather(
                                out_ap=gb[:, o // 128:(o + ni) // 128, :],
                                in_ap=src_ap,
                                idxs_ap=it[:, o // 16:(o + ni) // 16],
                                num_idxs=ni, num_idxs_reg=ni, elem_size=F,
                                queue_num=qctr[0] % NQ)
                            qctr[0] += 1
                        bufs[h] = (gb, g[f"b0{tg}"], nb)

                    # conv2: parity-select each gathered pair down to 64
                    # feats: gsel = gb[:, :, 0:64]*par + gb[:, :, 64:128]*parO
                    gsel = {}
                    if layer == 2:
                        for h, tg in ((0, "A"), (1, "B")):
                            if h not in bufs:
                                continue
                            gb, bbase, nb = bufs[h]
                            cb0 = g[f"b0{tg}"] + (0 if h == 0 else NBA)
                            gs = wseo_p.tile([128, GNBH, O], F16,
                                             tag=f"gs{tg}")
                            tmp = wseo_p.tile([128, GNBH, O], F16,
                                              tag=f"gt{tg}")
                            nc.vector.tensor_tensor(
                                out=gs[:, 0:nb, :],
                                in0=gb[:, 0:nb, 0:O],
                                in1=bcast_inner(par_sb[:, cb0:cb0 + nb], O),
                                op=mybir.AluOpType.mult)
                            nc.vector.tensor_tensor(
                                out=tmp[:, 0:nb, :],
                                in0=gb[:, 0:nb, O:2 * O],
                                in1=bcast_inner(parO_sb[:, cb0:cb0 + nb], O),
                                op=mybir.AluOpType.mult)
                            nc.vector.tensor_tensor(
                                out=gs[:, 0:nb, :], in0=gs[:, 0:nb, :],
                                in1=tmp[:, 0:nb, :],
                                op=mybir.AluOpType.add)
                            gsel[h] = gs

                    for t in range(t0, t1):
                        tn = 128 if t < T - 1 else LT
                        blist = []
                        for h in (0, 1):
                            if h not in bufs:
                                continue
                            gb, bbase, nb = bufs[h]
                            for b in range(int(tsb[h, t]) // 128,
                                           int(tsb[h, t + 1]) // 128):
                                cb = min(int(dends[h, b] - d0s[h, b]), Cu,
                                         128 - int(d0s[h, b]))
                                blist.append((h, gb, b - bbase,
                                              b + (0 if h == 0 else NBA),
                                              int(d0s[h, b]), cb))
                        pz = ptile.tile([128, 128], FP32, tag="pz")
                        n_mm = len(blist)
                        mi = 0
                        for (h, gb, j, cb_abs, dd0, cb) in blist:
                            po_ = int(offP[cb_abs])
                            if layer == 1:
                                nc.tensor.matmul(
                                    pz[:, dd0:dd0 + cb],
                                    lhsT=gb[:, j, :],
                                    rhs=wsP_sb[:, po_:po_ + cb],
                                    start=(mi == 0), stop=(mi == n_mm - 1),
                                    skip_group_check=True)
                            else:
                                nc.tensor.matmul(
                                    pz[:O, dd0:dd0 + cb],
                                    lhsT=gsel[h][:, j, :],
                                    rhs=wsP_sb[:, po_:po_ + cb],
                                    start=(mi == 0), stop=(mi == n_mm - 1),
                                    skip_group_check=True)
                            mi += 1
                        # close tile -> zT (f16) on Act (DVE is busier)
                        nc.scalar.activation(
                            out=zT[:HH, t * 128:t * 128 + tn],
                            in_=pz[:HH, :tn],
                            func=mybir.ActivationFunctionType.Identity)
                        # moments: transpose then M += z z^T, S += z^T 1
                        ptr = pmisc.tile([128, 128], F16, tag="ptr")
                        nc.tensor.transpose(ptr[:tn, :HH],
                                            zT[:HH, t * 128:t * 128 + tn],
                                            ident_h[:HH, :HH])
                        zd = small.tile([128, H + 1], F16, tag="zd")
                        if tn < 128:
                            nc.vector.memset(zd[:], 0.0)
                        nc.vector.memset(zd[:, HH:HH + 1], 1.0)
                        nc.scalar.activation(
                            out=zd[:tn, 0:HH], in_=ptr[:tn, :HH],
                            func=mybir.ActivationFunctionType.Identity)
                        nc.tensor.matmul(Mp[:HH, 0:HH + 1], lhsT=zd[:, 0:HH],
                                         rhs=zd[:, 0:HH + 1],
                                         start=(t == 0), stop=(t == T - 1),
                                         skip_group_check=True)
                mo_sb = small.tile([128, H + 1], FP32, tag="mo")
                nc.vector.tensor_copy(out=mo_sb[:HH], in_=Mp[:HH])
                if layer == 1:
                    min_d, mout_d = mo_in, mo_out
                    nc.sync.dma_start(out=min_d[:, :], in_=mo_sb[:])
                else:
                    min_d, mout_d = mo2_in, mo2_out
                    nc.sync.dma_start(out=min_d[:, :], in_=mo_sb[:O, 0:O + 1])
                nc.gpsimd.collective_compute(
                    "AllReduce", mybir.AluOpType.add, replica_groups=rg,
                    ins=[min_d.opt()], outs=[mout_d.opt()])
                mg = small.tile([128, H + 1], FP32, tag="mg")
                if layer == 1:
                    nc.sync.dma_start(out=mg[:], in_=mout_d[:, :])
                else:
                    nc.sync.dma_start(out=mg[:O, 0:O + 1], in_=mout_d[:, :])
                return mg

            def bn_fold1(mg):
                """layer1: y = zW1; scale/shift from moments folded thru W1."""
                pf = pfold.tile([128, 128], FP32, tag="pf")
                nc.tensor.matmul(pf[:H, 0:1], lhsT=W1_sb[:], rhs=mg[:, H:H + 1],
                                 start=True, stop=True, skip_group_check=True)
                mul_sb = small.tile([128, 1], FP32, tag="mul")
                nc.vector.tensor_scalar_mul(out=mul_sb[:H], in0=pf[:H, 0:1],
                                            scalar1=1.0 / N)
                pg = pfold.tile([128, 128], FP32, tag="pf")
                nc.tensor.matmul(pg[:, 0:H], lhsT=mg[:, 0:H], rhs=W1_sb[:],
                                 start=True, stop=True, skip_group_check=True)
                wg = small.tile([128, H], FP32, tag="wg")
                nc.vector.tensor_tensor(out=wg[:], in0=pg[:, 0:H], in1=W1_sb[:],
                                        op=mybir.AluOpType.mult)
                pd = pfold.tile([128, 128], FP32, tag="pf")
                nc.tensor.matmul(pd[:H, 0:1], lhsT=wg[:], rhs=ones_col[:],
                                 start=True, stop=True, skip_group_check=True)
                return finish_fold(pd, mul_sb, H, g1_sb, be1_sb)

            def bn_fold2(mg):
                """layer2: y = z + b2 (W2 pre-folded); direct moments."""
                mul_sb = small.tile([128, 1], FP32, tag="mul")
                nc.vector.tensor_scalar_mul(out=mul_sb[:O], in0=mg[:O, O:O + 1],
                                            scalar1=1.0 / N)
                wg = small.tile([128, O], FP32, tag="wg2")
                nc.vector.tensor_tensor(out=wg[:O], in0=mg[:O, 0:O],
                                        in1=ident[:O, :O],
                                        op=mybir.AluOpType.mult)
                pd = pfold.tile([128, 128], FP32, tag="pf")
                nc.tensor.matmul(pd[:O, 0:1], lhsT=wg[:O, :], rhs=ones_col[:O],
                                 start=True, stop=True, skip_group_check=True)
                return finish_fold(pd, mul_sb, O, g2_sb, be2_sb)

            def finish_fold(pd, mul_sb, HH, g_sb, be_sb):
                var_sb = small.tile([128, 1], FP32, tag="var")
                nc.vector.tensor_scalar_mul(out=var_sb[:HH], in0=pd[:HH, 0:1],
                                            scalar1=1.0 / N)
                mu2 = small.tile([128, 1], FP32, tag="mu2")
                nc.vector.tensor_mul(mu2[:HH], mul_sb[:HH], mul_sb[:HH])
                nc.vector.tensor_sub(var_sb[:HH], var_sb[:HH], mu2[:HH])
                sqv = small.tile([128, 1], FP32, tag="sqv")
                nc.scalar.activation(out=sqv[:HH], in_=var_sb[:HH],
                                     func=mybir.ActivationFunctionType.Sqrt,
                                     bias=eps_sb[:HH])
                s_sb = small.tile([128, 1], FP32, tag="s")
                nc.vector.reciprocal(out=s_sb[:HH], in_=sqv[:HH])
                nc.vector.tensor_mul(s_sb[:HH], s_sb[:HH], g_sb[:HH])
                tb_sb = small.tile([128, 1], FP32, tag="tb")
                nc.vector.tensor_mul(tb_sb[:HH], mul_sb[:HH], s_sb[:HH])
                nc.vector.tensor_sub(tb_sb[:HH], be_sb[:HH], tb_sb[:HH])
                return s_sb, tb_sb

            # ---- layer 1 ----
            if slev >= 1:
                zT1 = zpool.tile([128, NP], F16, tag="zbig")
                mg1 = conv(1, zT1, H)
                # y1 = W1^T zT1 (overlaps the moment AllReduce)
                y1sb = zpool.tile([128, NP], F16, tag="y1")
                for c0 in range(0, NP, YC):
                    c1 = min(c0 + YC, NP)
                    py = pyc.tile([128, YC], FP32, tag="py")
                    nc.tensor.matmul(py[:, 0:c1 - c0], lhsT=W1_16[:],
                                     rhs=zT1[:, c0:c1],
                                     start=True, stop=True, skip_group_check=True)
                    nc.vector.tensor_copy(out=y1sb[:, c0:c1], in_=py[:, 0:c1 - c0])
            if slev >= 2:
                s1, tb1 = bn_fold1(mg1)
                # u = LeakyReLU(BN(y1)); u2own = u @ W2
                for c0 in range(0, NP, YC):
                    c1 = min(c0 + YC, NP)
                    cw = c1 - c0
                    u = small.tile([128, YC], F16, tag="u")
                    nc.scalar.activation(out=u[:, 0:cw], in_=y1sb[:, c0:c1],
                                         func=mybir.ActivationFunctionType.Identity,
                                         scale=s1[:H], bias=tb1[:H])
                    v = small.tile([128, YC], F16, tag="v")
                    nc.vector.tensor_scalar_mul(out=v[:, 0:cw], in0=u[:, 0:cw],
                                                scalar1=NEG)
                    nc.vector.tensor_tensor(out=u[:, 0:cw], in0=u[:, 0:cw],
                                            in1=v[:, 0:cw], op=mybir.AluOpType.max)
                    p2 = pyc.tile([128, YC], FP32, tag="py")
                    nc.tensor.matmul(p2[:O, 0:cw], lhsT=W2_16[:], rhs=u[:, 0:cw],
                                     start=True, stop=True, skip_group_check=True)
                    u2sb = small.tile([128, YC], F16, tag="u2sb")
                    nc.vector.tensor_copy(out=u2sb[:O, 0:cw], in_=p2[:O, 0:cw])
                    for tb_ in range(c0 // 128, (c1 + 127) // 128):
                        n0 = tb_ * 128
                        tn = min(128, NP - n0)
                        po = pmisc.tile([128, 128], F16, tag="ptr")
                        nc.tensor.transpose(po[:tn, :O],
                                            u2sb[:O, n0 - c0:n0 - c0 + tn],
                                            ident_h[:O, :O])
                        xo = small.tile([128, O], F16, tag="xo")
                        nc.vector.tensor_copy(out=xo[:tn], in_=po[:tn, :O])
                        nc.sync.dma_start(out=u2own_d[n0:n0 + tn, :], in_=xo[:tn])
            if slev >= 3:
                nc.gpsimd.collective_compute(
                    "AllGather", mybir.AluOpType.bypass, replica_groups=rg,
                    ins=[u2own_d.opt()], outs=[u2full.opt()])

            # ---- layer 2 ----
            if slev >= 4:
                zT2 = zpool.tile([128, NP], F16, tag="zbig")
                mg2 = conv(2, zT2, O)
            if slev >= 5:
                s2, tb2 = bn_fold2(mg2)
                # out = BN(z2): scale feature-major (into the spent y1 tile),
                # transpose, write
                nc.scalar.activation(out=y1sb[:O, :], in_=zT2[:O, :],
                                     func=mybir.ActivationFunctionType.Identity,
                                     scale=s2[:O], bias=tb2[:O])
                for tb_ in range(T):
                    n0 = tb_ * 128
                    tn = min(128, NP - n0)
                    po = pmisc.tile([128, 128], F16, tag="ptr")
                    nc.tensor.transpose(po[:tn, :O], y1sb[:O, n0:n0 + tn],
                                        ident_h[:O, :O])
                    oo = small.tile([128, O], FP32, tag="oo")
                    nc.vector.tensor_copy(out=oo[:tn], in_=po[:tn, :O])
                    nc.sync.dma_start(out=out_d[n0:n0 + tn, :], in_=oo[:tn])

        for _rep in range(reps):
            emit_once()

        for p in (dram, pyc, pmom, pfold, pmisc, ptile, zpool, wseo_p,
                  gbufB_p, gbufA_p, small, sing):
            p.release()

    nc.compile()
    return nc


# ------------------------------------------------------------------ runner --
def make_in_maps(st, inputs):
    cfg = st["cfg"]
    NCORE = cfg["NCORE"]
    x16 = np.asarray(inputs["drug_smiles_fea"], np.float32).astype(np.float16)
    maps = []
    for c in range(NCORE):
        maps.append(dict(
            x16=x16,
            w1=np.asarray(inputs["W1"], np.float32),
            g1=np.asarray(inputs["g1"], np.float32),
            beta1=np.asarray(inputs["beta1"], np.float32),
            w2m=np.asarray(inputs["W2"], np.float32),
            g2=np.asarray(inputs["g2"], np.float32),
            beta2=np.asarray(inputs["beta2"], np.float32),
            idx1A=st["idx1A"][c], idx1B=st["idx1B"][c],
            idx2A=st["idx2A"][c], idx2B=st["idx2B"][c],
            wsP=np.ascontiguousarray(st["wsP"][c]),
            par=np.ascontiguousarray(st["par"][c]),
            parO=np.ascontiguousarray(st["parO"][c]),
        ))
    return maps


_LAST = {}


def kernel(**inputs):
    cfg = CFG
    adj = np.asarray(inputs["ATC_adj"])
    w = np.asarray(inputs["ATC_weight"], np.float32)
    st = preprocess(adj, w, cfg)
    nc = build(st)
    maps = make_in_maps(st, inputs)
    res = bass_utils.run_bass_kernel_spmd(
        nc, maps, core_ids=list(range(cfg["NCORE"])))
    out = np.concatenate([res.results[c]["out"] for c in range(cfg["NCORE"])], 0)
    _LAST.update(st=st, nc=nc, maps=maps)
    return out


# revision 19
# speedup vs baseline: 4.4758x; 4.4758x over previous
"""Trainium2 Bass kernel for nn_ATC_Network (2-layer GCN + BN + LeakyReLU).

Computation (see reference):
    row, col, w  (+ self loops w=1)
    deg[c] = sum_{e: col=c} w_e ;  dis = rsqrt(deg)
    norm_e = dis[row]*w*dis[col]
    z1[c]  = sum_e norm_e * x[row]          (conv1 aggregate, incl self loop)
    y1     = z1 @ W1 + b1 ; x2 = LeakyReLU(BN(y1))
    z2[c]  = sum_e norm_e * x2[row]
    y2     = z2 @ W2 + b2 ; out = BN(y2)

v2 design notes:
  - All graph normalization (deg, dis, norm) is computed host-side in
    preprocess(); self-loops become ordinary edges.  The device sees only
    padded slot streams + block tables.
  - x is cast to f16 host-side; conv1 gathers 256B rows straight from it.
  - The weighted one-hot ws ([128, NB, Cu] f16) is built once on-device
    (3 DVE passes) and shared by both conv layers.
  - conv2 pre-folds W2: aggregates u2 = x2@W2 (64-wide).  u2 is
    AllGather'd as f16 [N, 64] (6.4MB) and gathered in PAIRS (256B = two
    nodes); an even/odd split of ws (built per group) selects the right
    half of each gathered pair via two matmuls per block.
  - BN stats via second-moment matrices (PE) + AllReduce; the moment
    AllReduce overlaps the y1 = z1@W1 matmul.
"""

import sys

sys.path.insert(0, "/opt/trn_rl_repo")

import numpy as np
import ml_dtypes

import concourse.bass as bass
import concourse.tile as tile
from concourse import bacc, bass_utils, mybir
from concourse.masks import make_identity

FP32 = mybir.dt.float32
F16 = mybir.dt.float16
I16 = mybir.dt.int16

# ---------------------------------------------------------------- config ----
CFG = dict(
    N=50000, E=800000, F=128, H=128, O=64, NCORE=8,
    HALF=25000,          # src split point for int16 gather indices
    GT=4,                # dest tiles per gather group
    YC=512,              # node columns per y-matmul chunk
    EPS=1e-5, NEG=0.01,
)


# ---------------------------------------------------------- preprocessing ---
def preprocess(adj, w, cfg):
    """Host-side graph prep: norm, self-loops, per-core padded slot streams.

    Returns block tables with *uniform* shapes across cores (SPMD: one
    instruction stream; per-core data differs).
    """
    N, E, NCORE, HALF = cfg["N"], cfg["E"], cfg["NCORE"], cfg["HALF"]
    NP = N // NCORE
    T = (NP + 127) // 128
    LT = NP - 128 * (T - 1)

    row0 = np.asarray(adj[0], np.int64)
    col0 = np.asarray(adj[1], np.int64)
    w = np.asarray(w, np.float32)

    # gcn_norm host-side (incl self loops, weight 1)
    deg = np.zeros(N, np.float64)
    np.add.at(deg, col0, w.astype(np.float64))
    deg += 1.0
    dis = (1.0 / np.sqrt(deg)).astype(np.float32)

    row = np.concatenate([row0, np.arange(N, dtype=np.int64)])
    col = np.concatenate([col0, np.arange(N, dtype=np.int64)])
    nrm = np.concatenate([dis[row0] * w * dis[col0], dis * dis]).astype(np.float32)
    EA = E + N

    core = col // NP
    lc = col % NP
    tl = lc // 128
    dl = lc % 128
    half = (row >= HALF).astype(np.int64)

    # stable sort by (core, half, tile, dest)
    key = ((core * 2 + half) * T + tl) * 128 + dl
    order = np.argsort(key, kind="stable")

    # caps per (half, tile): max over cores, ceil to 128
    cht = (core * 2 + half) * T + tl
    cnt_cht = np.bincount(cht, minlength=NCORE * 2 * T).reshape(NCORE, 2, T)
    cap = np.maximum(cnt_cht.max(0), 1)                     # [2, T]
    cap = ((cap + 127) // 128) * 128
    tsb = np.zeros((2, T + 1), np.int64)                    # slot base per tile
    tsb[:, 1:] = np.cumsum(cap, 1)
    L = tsb[:, -1].copy()                                   # stream length/half
    NBA, NBB = int(L[0] // 128), int(L[1] // 128)
    NB = NBA + NBB

    # per-edge rank within (core, half, tile) -> stream slot
    sk2 = cht[order]
    g2_start = np.r_[0, np.flatnonzero(np.diff(sk2)) + 1]
    g2_id = np.cumsum(np.r_[0, np.diff(sk2) != 0])
    rank_in_ct = np.arange(EA) - g2_start[g2_id]
    slot = tsb[half[order], tl[order]] + rank_in_ct          # slot within half-stream

    # fill per-core streams (flat per (core, half))
    oc, oh = core[order], half[order]
    orow, onrm, odl = row[order], nrm[order], dl[order]
    idx1 = [[np.zeros(int(L[h]), np.int32) for h in range(2)] for _ in range(NCORE)]
    idx2 = [[np.zeros(int(L[h]), np.int32) for h in range(2)] for _ in range(NCORE)]
    n_st = [[np.zeros(int(L[h]), np.float32) for h in range(2)] for _ in range(NCORE)]
    d_st = [[np.zeros(int(L[h]), np.int32) for h in range(2)] for _ in range(NCORE)]
    p_st = [[np.zeros(int(L[h]), np.float32) for h in range(2)] for _ in range(NCORE)]
    for c in range(NCORE):
        for h in range(2):
            m = (oc == c) & (oh == h)
            s = slot[m]
            idx1[c][h][s] = orow[m] - h * HALF
            idx2[c][h][s] = orow[m] // 2
            n_st[c][h][s] = onrm[m]
            d_st[c][h][s] = odl[m]
            p_st[c][h][s] = 1.0 - (orow[m] & 1)              # even -> 1

    # uniform block structure: d0 / width per 128-slot block (union over cores)
    d0 = np.zeros((2, max(NBA, NBB)), np.int64)
    dend = np.ones((2, max(NBA, NBB)), np.int64)
    for h, nb in ((0, NBA), (1, NBB)):
        dmin = np.full((nb,), 1 << 30, np.int64)
        dmax = np.full((nb,), -1, np.int64)
        for c in range(NCORE):
            dd = d_st[c][h].reshape(nb, 128)
            ww = n_st[c][h].reshape(nb, 128)
            real = ww > 0
            any_real = real.any(1)
            dmasked = np.where(real, dd, 1 << 30)
            dmin2 = dmasked.min(1)
            dmasked = np.where(real, dd, -1)
            dmax2 = dmasked.max(1)
            dmin = np.where(any_real, np.minimum(dmin, dmin2), dmin)
            dmax = np.where(any_real, np.maximum(dmax, dmax2), dmax)
        none = dmax < 0
        dmin[none] = 0
        dmax[none] = 0
        wid = dmax + 1 - dmin
        d0[h, :nb] = dmin
        dend[h, :nb] = dmin + wid
    Cu = int((dend - d0).max())
    Cu = max(Cu, 2)

    # block -> tile map (per half)
    blk_tile = np.zeros((2, max(NBA, NBB)), np.int64)
    for h in range(2):
        for t in range(T):
            blk_tile[h, tsb[h, t] // 128: tsb[h, t + 1] // 128] = t

    # packed one-hot weight table: block b occupies cols offP[b]:offP[b]+cb_b
    cbs = np.zeros(NB, np.int64)
    cbs[:NBA] = np.minimum(np.minimum(dend[0, :NBA] - d0[0, :NBA], Cu),
                           128 - d0[0, :NBA])
    cbs[NBA:] = np.minimum(np.minimum(dend[1, :NBB] - d0[1, :NBB], Cu),
                           128 - d0[1, :NBB])
    offP = np.zeros(NB + 1, np.int64)
    offP[1:] = np.cumsum(cbs)
    P = int(((offP[-1] + 15) // 16) * 16)
    wsP = np.zeros((NCORE, 128, P), np.float16)
    for c in range(NCORE):
        for h, nb0 in ((0, 0), (1, NBA)):
            L_ = int(L[h])
            sl = np.arange(L_)
            bidx = sl // 128 + nb0
            p = sl % 128
            nrm_s = n_st[c][h]
            dls = d_st[c][h]
            colP = offP[bidx] + dls - (d0[h])[sl // 128]
            ok = nrm_s > 0
            wsP[c][p[ok], colP[ok]] = nrm_s[ok]

    # per-core block tables in [128, NB] layout
    def blockify(streams, dt):
        outa = np.zeros((NCORE, 128, NB), dt)
        for c in range(NCORE):
            outa[c, :, :NBA] = streams[c][0].reshape(NBA, 128).T
            outa[c, :, NBA:] = streams[c][1].reshape(NBB, 128).T
        return outa

    par = blockify(p_st, np.float32).astype(np.float16)
    parO = blockify([[1.0 - x for x in ps] for ps in p_st],
                    np.float32).astype(np.float16)

    # idx arrays, wrapped [16, L/16] then replicated to 128 partitions
    def wrap_idx(a):
        v = a.astype(np.int16).reshape(-1, 16).T  # [16, L/16]
        return np.tile(v, (8, 1))                 # [128, L/16]

    idx1A = np.stack([wrap_idx(idx1[c][0]) for c in range(NCORE)])
    idx1B = np.stack([wrap_idx(idx1[c][1]) for c in range(NCORE)])
    idx2A = np.stack([wrap_idx(idx2[c][0]) for c in range(NCORE)])
    idx2B = np.stack([wrap_idx(idx2[c][1]) for c in range(NCORE)])

    # gather groups (GT tiles each)
    GT = cfg["GT"]
    groups = []
    for g0 in range(0, T, GT):
        g1 = min(g0 + GT, T)
        ent = dict(t0=g0, t1=g1)
        for h, tag in ((0, "A"), (1, "B")):
            s0, s1 = int(tsb[h, g0]), int(tsb[h, g1])
            ent[f"s0{tag}"], ent[f"s1{tag}"] = s0, s1
            ent[f"b0{tag}"], ent[f"b1{tag}"] = s0 // 128, s1 // 128
        groups.append(ent)
    GNBH = max(max(g["b1A"] - g["b0A"], g["b1B"] - g["b0B"]) for g in groups)
    GNB = max((g["b1A"] - g["b0A"]) + (g["b1B"] - g["b0B"]) for g in groups)

    pad_ratio = (L[0] + L[1]) / max(1.0, EA / NCORE)
    return dict(
        cfg=cfg, NP=NP, T=T, LT=LT, NBA=NBA, NBB=NBB, NB=NB, Cu=Cu,
        LA=int(L[0]), LB=int(L[1]), tsb=tsb, d0=d0, dend=dend,
        blk_tile=blk_tile, groups=groups, GNBH=GNBH, GNB=GNB,
        wsP=wsP, offP=offP, cbs=cbs, P=P, par=par, parO=parO,
        idx1A=idx1A, idx1B=idx1B, idx2A=idx2A, idx2B=idx2B,
        pad_ratio=float(pad_ratio),
    )


# ------------------------------------------------------------ bass program --
STAGES = ["ws", "conv1", "bn1", "ag", "conv2", "full"]


def build(st, stage="full", reps=1):
    slev = STAGES.index(stage)
    cfg = st["cfg"]
    N, F, H, O, NCORE = cfg["N"], cfg["F"], cfg["H"], cfg["O"], cfg["NCORE"]
    HALF, EPS, NEG, YC = cfg["HALF"], cfg["EPS"], cfg["NEG"], cfg["YC"]
    NP, T, LT, NB, NBA, NBB, Cu = (st["NP"], st["T"], st["LT"], st["NB"],
                                   st["NBA"], st["NBB"], st["Cu"])
    d0s, dends, tsb = st["d0"], st["dend"], st["tsb"]
    GNBH, GNB = st["GNBH"], st["GNB"]
    P, offP, cbs = st["P"], st["offP"], st["cbs"]
    NH = N // 2  # pair count for conv2 gathers
    rg = [list(range(NCORE))]

    nc = bacc.Bacc("TRN2", target_bir_lowering=False, debug=False,
                   num_devices=NCORE, num_swdge_queues=4)
    NQ = 4
    qctr = [0]  # rotate swdge queues so desc-gen overlaps draining

    # --- I/O ---
    x16_d = nc.dram_tensor("x16", [N, F], F16, kind="ExternalInput")
    W1 = nc.dram_tensor("w1", [F, H], FP32, kind="ExternalInput")
    g1 = nc.dram_tensor("g1", [H], FP32, kind="ExternalInput")
    be1 = nc.dram_tensor("beta1", [H], FP32, kind="ExternalInput")
    W2 = nc.dram_tensor("w2m", [H, O], FP32, kind="ExternalInput")
    g2 = nc.dram_tensor("g2", [O], FP32, kind="ExternalInput")
    be2 = nc.dram_tensor("beta2", [O], FP32, kind="ExternalInput")
    i1A_d = nc.dram_tensor("idx1A", [128, st["LA"] // 16], I16, kind="ExternalInput")
    i1B_d = nc.dram_tensor("idx1B", [128, st["LB"] // 16], I16, kind="ExternalInput")
    i2A_d = nc.dram_tensor("idx2A", [128, st["LA"] // 16], I16, kind="ExternalInput")
    i2B_d = nc.dram_tensor("idx2B", [128, st["LB"] // 16], I16, kind="ExternalInput")
    wsP_d = nc.dram_tensor("wsP", [128, P], F16, kind="ExternalInput")
    par_d = nc.dram_tensor("par", [128, NB], F16, kind="ExternalInput")
    parO_d = nc.dram_tensor("parO", [128, NB], F16, kind="ExternalInput")
    out_d = nc.dram_tensor("out", [NP, O], FP32, kind="ExternalOutput")

    def bcast_inner(ap, k):
        return bass.AP(tensor=ap.tensor, offset=ap.offset, ap=ap.ap + [[0, k]])

    def bcast_mid(ap, k):
        # [128, n] -> [128, k, n] with middle stride 0
        return bass.AP(tensor=ap.tensor, offset=ap.offset,
                       ap=[ap.ap[0]] + [[0, k]] + ap.ap[1:])

    def bcast_part(src_ap, off, n, parts=128):
        return bass.AP(tensor=src_ap.tensor, offset=src_ap.offset + off,
                       ap=[[0, parts], [1, n]])

    with tile.TileContext(nc) as tc:
        sing = tc.alloc_tile_pool(name="sing", bufs=1)
        small = tc.alloc_tile_pool(name="small", bufs=3)
        gbufA_p = tc.alloc_tile_pool(name="gbufA", bufs=2)
        gbufB_p = tc.alloc_tile_pool(name="gbufB", bufs=2)
        wseo_p = tc.alloc_tile_pool(name="wseo", bufs=2)
        zpool = tc.alloc_tile_pool(name="zpool", bufs=1)
        ptile = tc.alloc_tile_pool(name="ptile", bufs=2, space="PSUM")
        pmisc = tc.alloc_tile_pool(name="pmisc", bufs=2, space="PSUM")
        pfold = tc.alloc_tile_pool(name="pfold", bufs=1, space="PSUM")
        pmom = tc.alloc_tile_pool(name="pmom", bufs=1, space="PSUM")
        pyc = tc.alloc_tile_pool(name="pyc", bufs=2, space="PSUM")
        dram = tc.alloc_tile_pool(name="dram", bufs=1, space="DRAM")

        # --- persistent DRAM scratch ---
        mo_in = dram.tile([128, H + 1], FP32)
        mo_out = dram.tile([128, H + 1], FP32)
        mo2_in = dram.tile([O, O + 1], FP32)
        mo2_out = dram.tile([O, O + 1], FP32)
        u2own_d = dram.tile([NP, O], F16)

        # --- constants ---
        ident = sing.tile([128, 128], FP32)
        make_identity(nc, ident[:])
        ident_h = sing.tile([128, 128], F16)
        nc.vector.tensor_copy(out=ident_h[:], in_=ident[:])
        ones_col = sing.tile([128, 1], FP32)
        nc.vector.memset(ones_col[:], 1.0)
        eps_sb = sing.tile([128, 1], FP32)
        nc.vector.memset(eps_sb[:], EPS)

        W1_sb = sing.tile([F, H], FP32)
        nc.sync.dma_start(out=W1_sb[:], in_=W1[:, :])
        W1_16 = sing.tile([F, H], F16)
        nc.vector.tensor_copy(out=W1_16[:], in_=W1_sb[:])
        W2_sb = sing.tile([H, O], FP32)
        nc.sync.dma_start(out=W2_sb[:], in_=W2[:, :])
        W2_16 = sing.tile([H, O], F16)
        nc.vector.tensor_copy(out=W2_16[:], in_=W2_sb[:])
        g1_sb = sing.tile([H, 1], FP32)
        nc.sync.dma_start(out=g1_sb[:], in_=g1[:, None])
        be1_sb = sing.tile([H, 1], FP32)
        nc.sync.dma_start(out=be1_sb[:], in_=be1[:, None])
        g2_sb = sing.tile([O, 1], FP32)
        nc.sync.dma_start(out=g2_sb[:], in_=g2[:, None])
        be2_sb = sing.tile([O, 1], FP32)
        nc.sync.dma_start(out=be2_sb[:], in_=be2[:, None])

        # --- block metadata ---
        wsP_sb = sing.tile([128, P], F16)
        nc.sync.dma_start(out=wsP_sb[:], in_=wsP_d[:, :])
        par_sb = sing.tile([128, NB], F16)
        nc.sync.dma_start(out=par_sb[:], in_=par_d[:, :])
        parO_sb = sing.tile([128, NB], F16)
        nc.sync.dma_start(out=parO_sb[:], in_=parO_d[:, :])

        def emit_once():
            # per-rep: Shared DRAM must be written by a single instruction
            u2full = dram.tile([N, O], F16, addr_space="Shared", tag=None,
                               uniquify=True, name="u2full")
            # =============== conv layer (shared emitter) ===============
            def conv(layer, zT, HH):
                """Aggregate into zT ([128, NP] f16, rows 0:HH valid); returns
                the AllReduce'd moment tile [128 or HH, HH+1]."""
                Mp = pmom.tile([128, H + 1], FP32, tag="mom")
                for g in st["groups"]:
                    t0, t1 = g["t0"], g["t1"]
                    bufs = {}
                    for h, tg, pool, idx_d in (
                            (0, "A", gbufA_p, i1A_d if layer == 1 else i2A_d),
                            (1, "B", gbufB_p, i1B_d if layer == 1 else i2B_d)):
                        s0, s1 = g[f"s0{tg}"], g[f"s1{tg}"]
                        nb = (s1 - s0) // 128
                        if nb == 0:
                            continue
                        it = small.tile([128, (s1 - s0) // 16], I16, tag=f"idx{tg}")
                        nc.sync.dma_start(out=it[:], in_=idx_d[:, s0 // 16:s1 // 16])
                        gb = pool.tile([128, GNBH, 128], F16, tag=f"g{tg}")
                        if layer == 1:
                            src_ap = (x16_d[0:HALF, :] if h == 0
                                      else x16_d[HALF:N, :])
                        else:
                            # pair view of u2full: [NH, 128] f16
                            u2ap = u2full[:]
                            src_ap = bass.AP(
                                tensor=u2ap.tensor, offset=u2ap.offset,
                                ap=[[128, NH], [1, 128]])
                        GCH = 1024
                        for o in range(0, s1 - s0, GCH):
                            ni = min(GCH, s1 - s0 - o)
                            nc.gpsimd.dma_gather(
                                out_ap=gb[:, o // 128:(o + ni) // 128, :],
                                in_ap=src_ap,
                                idxs_ap=it[:, o // 16:(o + ni) // 16],
                                num_idxs=ni, num_idxs_reg=ni, elem_size=F,
                                queue_num=qctr[0] % NQ)
                            qctr[0] += 1
                        bufs[h] = (gb, g[f"b0{tg}"], nb)

                    # conv2: parity-select each gathered pair down to 64
                    # feats: gsel = gb[:, :, 0:64]*par + gb[:, :, 64:128]*parO
                    gsel = {}
                    if layer == 2:
                        for h, tg in ((0, "A"), (1, "B")):
                            if h not in bufs:
                                continue
                            gb, bbase, nb = bufs[h]
                            cb0 = g[f"b0{tg}"] + (0 if h == 0 else NBA)
                            gs = wseo_p.tile([128, GNBH, O], F16,
                                             tag=f"gs{tg}")
                            tmp = wseo_p.tile([128, GNBH, O], F16,
                                              tag=f"gt{tg}")
                            nc.vector.tensor_tensor(
                                out=gs[:, 0:nb, :],
                                in0=gb[:, 0:nb, 0:O],
                                in1=bcast_inner(par_sb[:, cb0:cb0 + nb], O),
                                op=mybir.AluOpType.mult)
                            nc.vector.tensor_tensor(
                                out=tmp[:, 0:nb, :],
                                in0=gb[:, 0:nb, O:2 * O],
                                in1=bcast_inner(parO_sb[:, cb0:cb0 + nb], O),
                                op=mybir.AluOpType.mult)
                            nc.vector.tensor_tensor(
                                out=gs[:, 0:nb, :], in0=gs[:, 0:nb, :],
                                in1=tmp[:, 0:nb, :],
                                op=mybir.AluOpType.add)
                            gsel[h] = gs

                    for t in range(t0, t1):
                        tn = 128 if t < T - 1 else LT
                        blist = []
                        for h in (0, 1):
                            if h not in bufs:
                                continue
                            gb, bbase, nb = bufs[h]
                            for b in range(int(tsb[h, t]) // 128,
                                           int(tsb[h, t + 1]) // 128):
                                cb = min(int(dends[h, b] - d0s[h, b]), Cu,
                                         128 - int(d0s[h, b]))
                                blist.append((h, gb, b - bbase,
                                              b + (0 if h == 0 else NBA),
                                              int(d0s[h, b]), cb))
                        pz = ptile.tile([128, 128], FP32, tag="pz")
                        n_mm = len(blist)
                        mi = 0
                        for (h, gb, j, cb_abs, dd0, cb) in blist:
                            po_ = int(offP[cb_abs])
                            if layer == 1:
                                nc.tensor.matmul(
                                    pz[:, dd0:dd0 + cb],
                                    lhsT=gb[:, j, :],
                                    rhs=wsP_sb[:, po_:po_ + cb],
                                    start=(mi == 0), stop=(mi == n_mm - 1),
                                    skip_group_check=True)
                            else:
                                nc.tensor.matmul(
                                    pz[:O, dd0:dd0 + cb],
                                    lhsT=gsel[h][:, j, :],
                                    rhs=wsP_sb[:, po_:po_ + cb],
                                    start=(mi == 0), stop=(mi == n_mm - 1),
                                    skip_group_check=True)
                            mi += 1
                        # close tile -> zT (f16) on Act (DVE is busier)
                        nc.scalar.activation(
                            out=zT[:HH, t * 128:t * 128 + tn],
                            in_=pz[:HH, :tn],
                            func=mybir.ActivationFunctionType.Identity)
                        # moments: transpose then M += z z^T, S += z^T 1
                        ptr = pmisc.tile([128, 128], F16, tag="ptr")
                        nc.tensor.transpose(ptr[:tn, :HH],
                                            zT[:HH, t * 128:t * 128 + tn],
                                            ident_h[:HH, :HH])
                        zd = small.tile([128, H + 1], F16, tag="zd")
                        if tn < 128:
                            nc.vector.memset(zd[:], 0.0)
                        nc.vector.memset(zd[:, HH:HH + 1], 1.0)
                        nc.scalar.activation(
                            out=zd[:tn, 0:HH], in_=ptr[:tn, :HH],
                            func=mybir.ActivationFunctionType.Identity)
                        nc.tensor.matmul(Mp[:HH, 0:HH + 1], lhsT=zd[:, 0:HH],
                                         rhs=zd[:, 0:HH + 1],
                                         start=(t == 0), stop=(t == T - 1),
                                         skip_group_check=True)
                mo_sb = small.tile([128, H + 1], FP32, tag="mo")
                nc.vector.tensor_copy(out=mo_sb[:HH], in_=Mp[:HH])
                if layer == 1:
                    min_d, mout_d = mo_in, mo_out
                    nc.sync.dma_start(out=min_d[:, :], in_=mo_sb[:])
                else:
                    min_d, mout_d = mo2_in, mo2_out
                    nc.sync.dma_start(out=min_d[:, :], in_=mo_sb[:O, 0:O + 1])
                nc.gpsimd.collective_compute(
                    "AllReduce", mybir.AluOpType.add, replica_groups=rg,
                    ins=[min_d.opt()], outs=[mout_d.opt()])
                mg = small.tile([128, H + 1], FP32, tag="mg")
                if layer == 1:
                    nc.sync.dma_start(out=mg[:], in_=mout_d[:, :])
                else:
                    nc.sync.dma_start(out=mg[:O, 0:O + 1], in_=mout_d[:, :])
                return mg

            def bn_fold1(mg):
                """layer1: y = zW1; scale/shift from moments folded thru W1."""
                pf = pfold.tile([128, 128], FP32, tag="pf")
                nc.tensor.matmul(pf[:H, 0:1], lhsT=W1_sb[:], rhs=mg[:, H:H + 1],
                                 start=True, stop=True, skip_group_check=True)
                mul_sb = small.tile([128, 1], FP32, tag="mul")
                nc.vector.tensor_scalar_mul(out=mul_sb[:H], in0=pf[:H, 0:1],
                                            scalar1=1.0 / N)
                pg = pfold.tile([128, 128], FP32, tag="pf")
                nc.tensor.matmul(pg[:, 0:H], lhsT=mg[:, 0:H], rhs=W1_sb[:],
                                 start=True, stop=True, skip_group_check=True)
                wg = small.tile([128, H], FP32, tag="wg")
                nc.vector.tensor_tensor(out=wg[:], in0=pg[:, 0:H], in1=W1_sb[:],
                                        op=mybir.AluOpType.mult)
                pd = pfold.tile([128, 128], FP32, tag="pf")
                nc.tensor.matmul(pd[:H, 0:1], lhsT=wg[:], rhs=ones_col[:],
                                 start=True, stop=True, skip_group_check=True)
                return finish_fold(pd, mul_sb, H, g1_sb, be1_sb)

            def bn_fold2(mg):
                """layer2: y = z + b2 (W2 pre-folded); direct moments."""
                mul_sb = small.tile([128, 1], FP32, tag="mul")
                nc.vector.tensor_scalar_mul(out=mul_sb[:O], in0=mg[:O, O:O + 1],
                                            scalar1=1.0 / N)
                wg = small.tile([128, O], FP32, tag="wg2")
                nc.vector.tensor_tensor(out=wg[:O], in0=mg[:O, 0:O],
                                        in1=ident[:O, :O],
                                        op=mybir.AluOpType.mult)
                pd = pfold.tile([128, 128], FP32, tag="pf")
                nc.tensor.matmul(pd[:O, 0:1], lhsT=wg[:O, :], rhs=ones_col[:O],
                                 start=True, stop=True, skip_group_check=True)
                return finish_fold(pd, mul_sb, O, g2_sb, be2_sb)

            def finish_fold(pd, mul_sb, HH, g_sb, be_sb):
                var_sb = small.tile([128, 1], FP32, tag="var")
                nc.vector.tensor_scalar_mul(out=var_sb[:HH], in0=pd[:HH, 0:1],
                                            scalar1=1.0 / N)
                mu2 = small.tile([128, 1], FP32, tag="mu2")
                nc.vector.tensor_mul(mu2[:HH], mul_sb[:HH], mul_sb[:HH])
                nc.vector.tensor_sub(var_sb[:HH], var_sb[:HH], mu2[:HH])
                sqv = small.tile([128, 1], FP32, tag="sqv")
                nc.scalar.activation(out=sqv[:HH], in_=var_sb[:HH],
                                     func=mybir.ActivationFunctionType.Sqrt,
                                     bias=eps_sb[:HH])
                s_sb = small.tile([128, 1], FP32, tag="s")
                nc.vector.reciprocal(out=s_sb[:HH], in_=sqv[:HH])
                nc.vector.tensor_mul(s_sb[:HH], s_sb[:HH], g_sb[:HH])
                tb_sb = small.tile([128, 1], FP32, tag="tb")
                nc.vector.tensor_mul(tb_sb[:HH], mul_sb[:HH], s_sb[:HH])
                nc.vector.tensor_sub(tb_sb[:HH], be_sb[:HH], tb_sb[:HH])
                return s_sb, tb_sb

            # ---- layer 1 ----
            if slev >= 1:
                zT1 = zpool.tile([128, NP], F16, tag="zbig")
                mg1 = conv(1, zT1, H)
                # y1 = W1^T zT1 (overlaps the moment AllReduce)
                y1sb = zpool.tile([128, NP], F16, tag="y1")
                for c0 in range(0, NP, YC):
                    c1 = min(c0 + YC, NP)
                    py = pyc.tile([128, YC], FP32, tag="py")
                    nc.tensor.matmul(py[:, 0:c1 - c0], lhsT=W1_16[:],
                                     rhs=zT1[:, c0:c1],
                                     start=True, stop=True, skip_group_check=True)
                    nc.vector.tensor_copy(out=y1sb[:, c0:c1], in_=py[:, 0:c1 - c0])
            if slev >= 2:
                s1, tb1 = bn_fold1(mg1)
                # u = LeakyReLU(BN(y1)); u2own = u @ W2
                for c0 in range(0, NP, YC):
                    c1 = min(c0 + YC, NP)
                    cw = c1 - c0
                    u = small.tile([128, YC], F16, tag="u")
                    nc.scalar.activation(out=u[:, 0:cw], in_=y1sb[:, c0:c1],
                                         func=mybir.ActivationFunctionType.Identity,
                                         scale=s1[:H], bias=tb1[:H])
                    v = small.tile([128, YC], F16, tag="v")
                    nc.vector.tensor_scalar_mul(out=v[:, 0:cw], in0=u[:, 0:cw],
                                                scalar1=NEG)
                    nc.vector.tensor_tensor(out=u[:, 0:cw], in0=u[:, 0:cw],
                                            in1=v[:, 0:cw], op=mybir.AluOpType.max)
                    p2 = pyc.tile([128, YC], FP32, tag="py")
                    nc.tensor.matmul(p2[:O, 0:cw], lhsT=W2_16[:], rhs=u[:, 0:cw],
                                     start=True, stop=True, skip_group_check=True)
                    u2sb = small.tile([128, YC], F16, tag="u2sb")
                    nc.vector.tensor_copy(out=u2sb[:O, 0:cw], in_=p2[:O, 0:cw])
                    for tb_ in range(c0 // 128, (c1 + 127) // 128):
                        n0 = tb_ * 128
                        tn = min(128, NP - n0)
                        po = pmisc.tile([128, 128], F16, tag="ptr")
                        nc.tensor.transpose(po[:tn, :O],
                                            u2sb[:O, n0 - c0:n0 - c0 + tn],
                                            ident_h[:O, :O])
                        xo = small.tile([128, O], F16, tag="xo")
                        nc.vector.tensor_copy(out=xo[:tn], in_=po[:tn, :O])
                        nc.sync.dma_start(out=u2own_d[n0:n0 + tn, :], in_=xo[:tn])
            if slev >= 3:
                nc.gpsimd.collective_compute(
                    "AllGather", mybir.AluOpType.bypass, replica_groups=rg,
                    ins=[u2own_d.opt()], outs=[u2full.opt()])

            # ---- layer 2 ----
            if slev >= 4:
                zT2 = zpool.tile([128, NP], F16, tag="zbig")
                mg2 = conv(2, zT2, O)
            if slev >= 5:
                s2, tb2 = bn_fold2(mg2)
                # out = BN(z2): scale feature-major (into the spent y1 tile),
                # transpose, write
                nc.scalar.activation(out=y1sb[:O, :], in_=zT2[:O, :],
                                     func=mybir.ActivationFunctionType.Identity,
                                     scale=s2[:O], bias=tb2[:O])
                for tb_ in range(T):
                    n0 = tb_ * 128
                    tn = min(128, NP - n0)
                    po = pmisc.tile([128, 128], F16, tag="ptr")
                    nc.tensor.transpose(po[:tn, :O], y1sb[:O, n0:n0 + tn],
                                        ident_h[:O, :O])
                    oo = small.tile([128, O], FP32, tag="oo")
                    nc.vector.tensor_copy(out=oo[:tn], in_=po[:tn, :O])
                    nc.sync.dma_start(out=out_d[n0:n0 + tn, :], in_=oo[:tn])

        for _rep in range(reps):
            emit_once()

        for p in (dram, pyc, pmom, pfold, pmisc, ptile, zpool, wseo_p,
                  gbufB_p, gbufA_p, small, sing):
            p.release()

    nc.compile()
    return nc


# ------------------------------------------------------------------ runner --
def make_in_maps(st, inputs):
    cfg = st["cfg"]
    NCORE = cfg["NCORE"]
    x16 = np.asarray(inputs["drug_smiles_fea"], np.float32).astype(np.float16)
    maps = []
    for c in range(NCORE):
        maps.append(dict(
            x16=x16,
            w1=np.asarray(inputs["W1"], np.float32),
            g1=np.asarray(inputs["g1"], np.float32),
            beta1=np.asarray(inputs["beta1"], np.float32),
            w2m=np.asarray(inputs["W2"], np.float32),
            g2=np.asarray(inputs["g2"], np.float32),
            beta2=np.asarray(inputs["beta2"], np.float32),
            idx1A=st["idx1A"][c], idx1B=st["idx1B"][c],
            idx2A=st["idx2A"][c], idx2B=st["idx2B"][c],
            wsP=np.ascontiguousarray(st["wsP"][c]),
            par=np.ascontiguousarray(st["par"][c]),
            parO=np.ascontiguousarray(st["parO"][c]),
        ))
    return maps


_LAST = {}


def kernel(**inputs):
    cfg = CFG
    adj = np.asarray(inputs["ATC_adj"])
    w = np.asarray(inputs["ATC_weight"], np.float32)
    st = preprocess(adj, w, cfg)
    nc = build(st)
    maps = make_in_maps(st, inputs)
    res = bass_utils.run_bass_kernel_spmd(
        nc, maps, core_ids=list(range(cfg["NCORE"])))
    out = np.concatenate([res.results[c]["out"] for c in range(cfg["NCORE"])], 0)
    _LAST.update(st=st, nc=nc, maps=maps)
    return out
